# revision 30
# baseline (speedup 1.0000x reference)
"""CharLSTM (B=128, T=256, V=256, D=1024, L=4) on 8 trn2 NeuronCores.

Tensor-parallel over the 4*D gate dimension: core j owns, for each gate
m in {i,f,g,o}, columns [m*1024 + j*128 : m*1024 + (j+1)*128].  Hence
core j also owns h/c slice j*128:(j+1)*128 of the hidden dim.

Batch-major compute layout (activation-stationary matmuls): per layer
step, z[b, 512] = sum_k hT_chunk[k].T @ W_chunk[k] with N=512 moving
free dim, so the PE streams 512 cols per instruction (8 MMs per part
instead of 32 at N=128).  The LSTM cell runs elementwise on [b, gate]
tiles; the produced h slice [b,128] is transposed to [128 d, b] via the
DMA XBAR (off the PE), staged to DRAM, and AllGathered in layer pairs
(0,1) and (2,3) so every core has the full hT for the next step.

Layer l at time t runs at tick t + SKEW*l (wavefront), which gives the
x-part consumers >=1 tick of slack so AllGather latency hides under
matmuls of other layers.

Layer-0's x-part contracts a host-built one-hot over V=256 against
G0 = embed @ Wx[0] (computed on device), i.e. 2 matmuls instead of 8.

Output projection h3 @ Wout is computed redundantly on every core per
tick; the host reads core 0's copy.
"""

import numpy as np
import ml_dtypes

B, T, V, D, L = 128, 256, 256, 1024, 4
NCORES = 8
SKEW = 2
S = 4  # hbuf time slots
BF16 = ml_dtypes.bfloat16
TMODE = "pe"  # "xbar" (DMA transpose) or "pe" (tensor-engine transpose)


def _host_prep(idx, embed, Wx, Wh, b, Wout, t_run):
    """Build per-core input maps (numpy)."""
    nw = (t_run + 3) // 4
    idx = np.asarray(idx)
    embed = np.asarray(embed, np.float32)
    Wx = np.asarray(Wx, np.float32)
    Wh = np.asarray(Wh, np.float32)
    b = np.asarray(b, np.float32)
    Wout = np.asarray(Wout, np.float32)
    assert not np.any(b), "nonzero bias not supported by this build"

    # embt[p, k, v] = embed[v, k*128+p]
    embt = np.ascontiguousarray(
        embed.T.reshape(8, 128, V).transpose(1, 0, 2)).astype(BF16)
    # wout[p, k, v] = Wout[k*128+p, v]
    wout = np.ascontiguousarray(
        Wout.reshape(8, 128, V).transpose(1, 0, 2)).astype(BF16)
    # one-hot: oh[w, p, c, kk, bb] = (idx[bb, 4w+kk] == c*128+p)
    ids = idx[:, :t_run]  # [B, t_run]
    onehot = (ids[None, :, :] == np.arange(V)[:, None, None])  # [V, B, t]
    oh_full = onehot.reshape(2, 128, B, nw, 4)  # [c, p, b, w, kk]
    oh = np.ascontiguousarray(
        oh_full.transpose(3, 1, 0, 4, 2)).astype(BF16)  # [w, p, c, kk, bb]

    # weight rhs layout: w[l, p, k, mm*128+c] = W[l, k*128+p, m*1024+j*128+c]
    # with gate order [i, f, o, g] (one fused 384-wide sigmoid over i,f,o)
    MSEL = [0, 1, 3, 2]
    wx_full = Wx.reshape(L, 8, 128, 4, 8, 128)[:, :, :, MSEL]  # [l, k, p, mm, j, c]
    wh_full = Wh.reshape(L, 8, 128, 4, 8, 128)[:, :, :, MSEL]

    in_maps = []
    for j in range(NCORES):
        wx_j = np.ascontiguousarray(
            wx_full[:, :, :, :, j, :].transpose(0, 2, 1, 3, 4)
        ).reshape(L, 128, 8, 512).astype(BF16)
        wh_j = np.ascontiguousarray(
            wh_full[:, :, :, :, j, :].transpose(0, 2, 1, 3, 4)
        ).reshape(L, 128, 8, 512).astype(BF16)
        in_map = {
            "wx": wx_j,
            "wh": wh_j,
            "embt": embt,
            "wout": wout,
            "oh": oh,
        }
        if TMODE == "pe":
            in_map["ident"] = np.eye(128, dtype=BF16)
        in_maps.append(in_map)
    return in_maps


def _build(nc, tile, mybir, t_run):
    """Emit the SPMD program for one core (identical on all cores)."""
    import bass_rust
    add_dep = bass_rust.add_dep_helper
    dt = mybir.dt
    nw = (t_run + 3) // 4

    wx_ext = nc.dram_tensor("wx", [L, 128, 8, 512], dt.bfloat16, kind="ExternalInput")
    wh_ext = nc.dram_tensor("wh", [L, 128, 8, 512], dt.bfloat16, kind="ExternalInput")
    embt_ext = nc.dram_tensor("embt", [128, 8, V], dt.bfloat16, kind="ExternalInput")
    wout_ext = nc.dram_tensor("wout", [128, 8, V], dt.bfloat16, kind="ExternalInput")
    oh_ext = nc.dram_tensor("oh", [nw, 128, 2, 4, 128], dt.bfloat16, kind="ExternalInput")
    if TMODE == "pe":
        ident_ext = nc.dram_tensor("ident", [128, 128], dt.bfloat16, kind="ExternalInput")
    out_ext = nc.dram_tensor("logits", [t_run, 128, V], dt.float32, kind="ExternalOutput")

    rg = [list(range(NCORES))]
    total_ticks = t_run + SKEW * (L - 1) + 1  # last tick flushes last proj

    with tile.TileContext(nc) as tc:
        with (
            tc.tile_pool(name="const", bufs=1) as cpool,
            tc.tile_pool(name="state", bufs=1) as spool,
            tc.tile_pool(name="work", bufs=6) as wpool,
            tc.tile_pool(name="ohp", bufs=2) as ohpool,
            tc.tile_pool(name="psum", bufs=6, space="PSUM") as psum,
            tc.tile_pool(name="ccin", bufs=3, space="DRAM") as ccin_pool,
            tc.tile_pool(name="ccout", bufs=3, space="DRAM") as ccout_pool,
        ):
            # ---- resident tiles ----
            wx_s = cpool.tile([128, L, 8, 512], dt.bfloat16)
            wh_s = cpool.tile([128, L, 8, 512], dt.bfloat16)
            embt_s = cpool.tile([128, 8, V], dt.bfloat16)
            wout_s = cpool.tile([128, 8, V], dt.bfloat16)
            g0_s = cpool.tile([128, 2, 512], dt.bfloat16)
            # (d, k-chunk, slot, l, b): l innermost-but-b so a pair's
            # scatter collapses to [p, k, (l b)] with 512B segments
            hbuf = spool.tile([128, 8, S, L, 128], dt.bfloat16)
            c_s = spool.tile([128, L, 128], dt.float32)

            for l in range(L):
                nc.sync.dma_start(wx_s[:, l], wx_ext[l])
                nc.sync.dma_start(wh_s[:, l], wh_ext[l])
            nc.sync.dma_start(embt_s[:], embt_ext[:])
            nc.sync.dma_start(wout_s[:], wout_ext[:])
            if TMODE == "pe":
                ident = cpool.tile([128, 128], dt.bfloat16)
                nc.sync.dma_start(ident[:], ident_ext[:])

            # ---- G0 = embed @ Wx[0] (slice), bf16, [v_chunk 128, 512] ----
            for c in range(2):
                pg = psum.tile([128, 512], dt.float32, tag="z", name="pg")
                for k in range(8):
                    nc.tensor.matmul(
                        pg[:], embt_s[:, k, c * 128:(c + 1) * 128], wx_s[:, 0, k],
                        start=(k == 0), stop=(k == 7),
                    )
                nc.vector.tensor_copy(g0_s[:, c], pg[:])

            # ---- main loop: software-pipelined emission ----
            # Emission block EB(tau) emits, in intended PE order:
            #   scatter-A(tau-1) | h0(tau) h1(tau) cells | T0 | x0,x1(tau+1) |
            #   T1 + AG-A(tau) | x2,x3(tau+1) | scatter-B(tau-1) | proj(tau) |
            #   h2(tau) cell2 | h3(tau) cell3 T2 | T3 + AG-B(tau)
            # so next tick's x-parts fill the AllGather shadow and AG-gated
            # h-parts never head-of-line-block independent work.
            oh_tiles = {}
            pend_a, pend_b = [], []  # deferred hbuf scatters (prev tick's AGs)

            def emit_scatters(jobs, eng):
                for ccout_p, lps, prod in jobs:
                    # ccout[r, p, i, b] -> hbuf[p, r, prod%S, lp, b]
                    lo, hi = lps[0], lps[-1]
                    eng.dma_start(
                        hbuf[:, :, prod % S, lo:hi + 1, :],
                        ccout_p[:, :, lo % 2:hi % 2 + 1, :].transpose(
                            (1, 0, 2, 3)),
                    )
                jobs.clear()

            def actives(tau):
                return [(l, tau - SKEW * l) for l in range(L)
                        if 0 <= tau - SKEW * l < t_run]

            def phase_a(tau_a, lo, hi, zps, barrier):
                """Emit x-part MM groups for layers lo..hi-1 of tick tau_a."""
                for l, t in actives(tau_a):
                    if not (lo <= l < hi):
                        continue
                    if l == 0:
                        for w in range(nw):
                            if max(0, w * 4 - 1) == t:
                                oht = ohpool.tile([128, 2, 4, 128], dt.bfloat16,
                                                  name="oh_t")
                                nc.sync.dma_start(oht[:], oh_ext[w])
                                oh_tiles[w] = oht
                        oh_t = oh_tiles[t // 4]
                    zp = psum.tile([128, 512], dt.float32, name="zp", tag="z")
                    zps[l] = zp
                    if l == 0:
                        for c in range(2):
                            barrier = nc.tensor.matmul(
                                zp[:], oh_t[:, c, t % 4, :], g0_s[:, c],
                                start=(c == 0), stop=(t == 0 and c == 1),
                            )
                    else:
                        # h^{l-1}_t was produced at tick t+SKEW*(l-1)
                        s = (t + SKEW * (l - 1)) % S
                        for k in range(8):
                            barrier = nc.tensor.matmul(
                                zp[:], hbuf[:, k, s, l - 1, :], wx_s[:, l, k],
                                start=(k == 0), stop=(t == 0 and k == 7),
                            )
                return barrier

            def h_part_and_cell(l, t, zp, barrier):
                if t > 0:
                    # h^l_{t-1} was produced at tick t-1+SKEW*l
                    s = (t - 1 + SKEW * l) % S
                    for k in range(8):
                        mm = nc.tensor.matmul(
                            zp[:], hbuf[:, k, s, l, :], wh_s[:, l, k],
                            start=False, stop=(k == 7),
                        )
                        if k == 0 and barrier is not None:
                            add_dep(mm.ins, barrier.ins, sync=False,
                                    reason="x before h on PE")
                # ---- LSTM cell elementwise ([b, gate] layout) ----
                # z slices: i=[0:128] f=[128:256] o=[256:384] g=[384:512]
                sig = wpool.tile([128, 384], dt.float32, name="sig")
                tg = wpool.tile([128, 128], dt.float32, name="tg")
                nc.scalar.activation(
                    sig[:, 0:384], zp[:, 0:384],
                    mybir.ActivationFunctionType.Sigmoid)
                nc.scalar.activation(
                    tg[:], zp[:, 384:512], mybir.ActivationFunctionType.Tanh)
                ig = wpool.tile([128, 128], dt.float32, name="ig")
                nc.vector.tensor_mul(ig[:], sig[:, 0:128], tg[:])
                cv = c_s[:, l]
                if t > 0:
                    nc.vector.tensor_mul(cv, cv, sig[:, 128:256])
                    nc.vector.tensor_add(cv, cv, ig[:])
                else:
                    nc.vector.tensor_copy(cv, ig[:])
                tch = wpool.tile([128, 128], dt.float32, name="tch")
                nc.scalar.activation(tch[:], cv, mybir.ActivationFunctionType.Tanh)
                h_sl = wpool.tile([128, 128], dt.bfloat16, name="h_sl")
                nc.vector.tensor_mul(h_sl[:], sig[:, 256:384], tch[:])
                return h_sl

            zps_cur = {}
            barrier = phase_a(0, 0, L, zps_cur, None)  # prologue: x-parts(0)

            for tau in range(total_ticks):
                act = actives(tau)
                act_map = dict(act)
                A_layers = [(l, t) for l, t in act if l < 2]
                B_layers = [(l, t) for l, t in act if l >= 2]
                lastA = A_layers[-1][0] if A_layers else None
                lastB = B_layers[-1][0] if B_layers else None
                zps, zps_cur = zps_cur, {}
                bar_prev, barrier = barrier, None

                ccinA = ccin_pool.tile([128, 2, 128], dt.bfloat16, name="ccinA")
                ccinB = ccin_pool.tile([128, 2, 128], dt.bfloat16, name="ccinB")
                ccoutA = ccout_pool.tile([NCORES, 128, 2, 128], dt.bfloat16,
                                         addr_space="Shared", name="ccoutA")
                ccoutB = ccout_pool.tile([NCORES, 128, 2, 128], dt.bfloat16,
                                         addr_space="Shared", name="ccoutB")

                h_tiles = {}
                hTA = wpool.tile([128, 2, 128], dt.bfloat16, name="hTA")
                hTB = wpool.tile([128, 2, 128], dt.bfloat16, name="hTB")

                def stage(l):
                    """Transpose h slice l to [d, b] into the pair's staging
                    tile; on the pair's last layer, DMA the pair to DRAM in
                    one shot and post its AllGather."""
                    with tc.high_priority():
                        hT_pair = hTA if l < 2 else hTB
                        pt = psum.tile([128, 128], dt.bfloat16, name="pt",
                                       tag="pp", bufs=2)
                        nc.tensor.transpose(pt[:], h_tiles[l][:], ident[:])
                        nc.vector.tensor_copy(hT_pair[:, l % 2], pt[:])
                        if l == lastA or l == lastB:
                            ccin_p = ccinA if l == lastA else ccinB
                            ccout_p = ccoutA if l == lastA else ccoutB
                            hT_pair = hTA if l == lastA else hTB
                            lps = [lp for lp in
                                   ((0, 1) if l == lastA else (2, 3))
                                   if lp in act_map]
                            lo, hi = lps[0] % 2, lps[-1] % 2
                            nc.sync.dma_start(
                                ccin_p[:, lo:hi + 1, :],
                                hT_pair[:, lo:hi + 1, :])
                            nc.gpsimd.collective_compute(
                                "AllGather", mybir.AluOpType.bypass,
                                replica_groups=rg, ins=[ccin_p[:]],
                                outs=[ccout_p[:]],
                            )
                            pend = pend_a if l == lastA else pend_b
                            pend.append((ccout_p, lps, tau))

                # 1. pair-A scatters from last tick (AG-A long since complete)
                emit_scatters(pend_a, nc.sync)

                # 2. A-pair layers: h-part + cell; stage first A layer late
                for i, (l, t) in enumerate(A_layers):
                    h_tiles[l] = h_part_and_cell(l, t, zps[l], bar_prev)
                    if i >= 1:
                        stage(A_layers[0][0])

                # 3. next tick's x-parts, layers 0-1 (fills AG shadow)
                barrier = phase_a(tau + 1, 0, 2, zps_cur, barrier)

                # 4. stage last A layer -> posts AG-A(tau)
                if A_layers:
                    if len(A_layers) == 1:
                        stage(A_layers[0][0])
                    else:
                        stage(lastA)

                # 5. next tick's x-part layer 2; then pair-B scatters from
                # last tick (they write the layer-2 slot x3 reads and the
                # layer-3 slots proj/h2/h3 read); then x-part layer 3
                barrier = phase_a(tau + 1, 2, 3, zps_cur, barrier)
                emit_scatters(pend_b, nc.scalar)
                barrier = phase_a(tau + 1, 3, L, zps_cur, barrier)
                t3p = tau - SKEW * 3 - 1
                if 0 <= t3p < t_run:
                    pp = psum.tile([128, V], dt.float32, name="pp", tag="pp",
                                   bufs=2)
                    s3 = (t3p + SKEW * 3) % S
                    for k in range(8):
                        mm = nc.tensor.matmul(
                            pp[:], hbuf[:, k, s3, 3, :], wout_s[:, k],
                            start=(k == 0), stop=(k == 7),
                        )
                        if k == 0 and bar_prev is not None:
                            add_dep(mm.ins, bar_prev.ins, sync=False,
                                    reason="x before proj on PE")
                    lg = wpool.tile([128, V], dt.float32, name="lg")
                    nc.vector.tensor_copy(lg[:], pp[:])
                    nc.scalar.dma_start(out_ext[t3p], lg[:])

                # 7. B-pair layers: h-part + cell; stage first B layer late
                for i, (l, t) in enumerate(B_layers):
                    h_tiles[l] = h_part_and_cell(l, t, zps[l], bar_prev)
                    if i >= 1:
                        stage(B_layers[0][0])
                if B_layers:
                    if len(B_layers) == 1:
                        stage(B_layers[0][0])
                    else:
                        stage(lastB)

    nc.compile()
    return nc


_CACHED = {}


def _get_nc(t_run):
    if t_run in _CACHED:
        return _CACHED[t_run]
    import concourse.bass as bass  # noqa: PLC0415
    import concourse.tile as tile  # noqa: PLC0415
    from concourse import bacc, mybir  # noqa: PLC0415

    nc = bacc.Bacc("TRN2", target_bir_lowering=False, debug=False,
                   num_devices=NCORES)
    _build(nc, tile, mybir, t_run)
    _CACHED[t_run] = nc
    return nc


def _postprocess(out, t_run):
    # out: [t, b, v] fp32 -> [B, t, V]
    return np.ascontiguousarray(
        np.asarray(out).transpose(1, 0, 2)).astype(np.float32)


def kernel(idx, embed, Wx, Wh, b, Wout, _t_run=T):
    from concourse.bass_utils import run_bass_kernel_spmd  # noqa: PLC0415

    t_run = _t_run
    in_maps = _host_prep(idx, embed, Wx, Wh, b, Wout, t_run)
    nc = _get_nc(t_run)
    res = run_bass_kernel_spmd(nc, in_maps, core_ids=list(range(NCORES)))
    return _postprocess(res.results[0]["logits"], t_run)
